# revision 9
# baseline (speedup 1.0000x reference)
"""Trainium2 Bass kernel for nn_GatherModel (NNConv GNN message passing).

8-core SPMD, edge-parallel sharded by destination node block.

v3 design: per-edge weight matrices W' = h_e @ en2_w' ([E, 42*42]) are built
once on device in bf16 (halving the baseline's HBM traffic) and streamed each
of the 6 message-passing steps into SBUF, where a custom Vector-engine
multiply+prefix-scan op contracts them with gathered source features. The en2
bias never enters W': the scatter matmul aggregates both messages and raw
source features (S), and each window epilogue adds S @ B with one small
matmul (bias-via-scatter, exact by linearity).
  - scatter (segment-sum over dst) is a PE matmul against precomputed one-hot
    window matrices (SBUF-resident bf16, built once in setup)
  - node update runs in fp32 transposed feature layout; updated features are
    cast to bf16 rows and AllGather'd across the 8 cores each step
"""
import numpy as np

import concourse.bacc as bacc
import concourse.bass as bass
import concourse.mybir as mybir
import concourse.tile as tile
from concourse import bass_utils, dve_ops
from concourse.dve_spec import Spec, Src0, Src1, scan, AluOp, lower, _has_src1
from concourse.dve_uop import DveOpSpec

N = 50000
E = 150000
D_IN = 42
D_H = 42
E_IN = 10
E_H = 128
STEPS = 6
N_CORES = 8
NPC = N // N_CORES          # 6250 nodes per core
WIN = 128                   # scatter window (node block) size
N_WIN = (NPC + WIN - 1) // WIN  # 49 windows per core, last partial (106)
NW = D_H * D_H              # 1764
CHUNK = 441                 # W-build matmul chunk (1 PSUM bank holds 512 fp32)
F32 = mybir.dt.float32
BF16 = mybir.dt.bfloat16
I32 = mybir.dt.int32


def _register_prefix_mac():
    name = "PREFIX_MAC_GNN"
    if name in dve_ops._SUB_OPCODE_FOR_NAME:
        return next(op for op in dve_ops.OPS if op.name == name)
    spec = Spec(
        body=scan(AluOp.ADD, Src0 * Src1),
        reference=lambda in0, in1, s0, s1, imm2: np.cumsum(
            (in0.astype(np.float32) * in1).reshape(in0.shape[0], -1), axis=-1
        ),
    )
    shas = {}
    row = dve_ops._CUSTOM_DVE_ROW_BASE + len(dve_ops.OPS)
    for ver in ("v3", "v4"):
        uops = lower(spec, ver=ver)
        shas[ver] = DveOpSpec(name=name, opcode=row, uops=uops,
                              rd1_en=_has_src1(spec)).sha(ver)
    op = dve_ops.DveOp(name, spec, subdim=False, uops_sha=shas)
    dve_ops.OPS.append(op)
    dve_ops._SUB_OPCODE_FOR_NAME[name] = row
    dve_ops.CUSTOM_DVE_SPECS[name] = spec
    return op


def _host_prep(n_feat, e_feat, src, dst):
    """Sort edges by dst, shard by dst block, pad each (core, window) edge run
    onto a shared slot grid so the tile->window map is identical on all cores."""
    order = np.argsort(dst, kind="stable")
    src_s, dst_s, ef_s = src[order], dst[order], e_feat[order]

    # per (core, window) counts
    core_e = dst_s // NPC
    loc = dst_s - core_e * NPC
    win_e = loc // WIN
    cnt = np.zeros((N_CORES, N_WIN), dtype=np.int64)
    np.add.at(cnt, (core_e, win_e), 1)

    slot_cnt = cnt.max(axis=0)                       # shared grid
    G = np.concatenate([[0], np.cumsum(slot_cnt)])   # window slot boundaries
    total = int(G[-1])
    T = (total + 127) // 128                         # edge tiles per core
    E_PAD = T * 128

    # per-core padded edge arrays
    src_pad = np.zeros((N_CORES, E_PAD), dtype=np.int32)
    dstrel_pad = np.full((N_CORES, E_PAD), -1.0, dtype=np.float32)
    ef_pad = np.zeros((N_CORES, E_PAD, E_IN), dtype=np.float32)

    # tile -> window band
    w0 = np.zeros(T, dtype=np.int64)       # first window overlapping tile t
    bw = np.zeros(T, dtype=np.int64)       # how many windows overlap tile t
    for t in range(T):
        lo, hi = t * 128, min((t + 1) * 128, total)
        wlo = int(np.searchsorted(G, lo, side="right") - 1)
        whi = int(np.searchsorted(G, max(hi - 1, lo), side="right") - 1)
        wlo, whi = min(wlo, N_WIN - 1), min(whi, N_WIN - 1)
        w0[t] = wlo
        bw[t] = whi - wlo + 1
    B_W = int(bw.max())

    # fill padded arrays: window w of core k occupies slots [G[w], G[w]+cnt[k,w])
    core_starts = np.searchsorted(core_e, np.arange(N_CORES))
    for k in range(N_CORES):
        base = core_starts[k]
        cw = np.concatenate([[0], np.cumsum(cnt[k])])
        for w in range(N_WIN):
            s0, s1 = int(base + cw[w]), int(base + cw[w + 1])
            g0 = int(G[w])
            n_e = s1 - s0
            src_pad[k, g0:g0 + n_e] = src_s[s0:s1]
            ef_pad[k, g0:g0 + n_e] = ef_s[s0:s1]
            # dst_rel relative to the band anchor of the edge's tile
            slots = np.arange(g0, g0 + n_e)
            dstrel_pad[k, g0:g0 + n_e] = (
                loc[s0:s1] - w0[slots // 128] * WIN).astype(np.float32)

    # scatter pair list (t, w) from actual overlap, and per-window tile ranges
    pairs = []
    for t in range(T):
        for j in range(int(bw[t])):
            w = int(w0[t]) + j
            if w < N_WIN:
                pairs.append((t, w))
    win_tiles = {w: [t for (t, ww) in pairs if ww == w] for w in range(N_WIN)}

    # offset of each tile's one-hot block inside the resident oh buffer
    oh_off = np.zeros(T + 1, dtype=np.int64)
    for t in range(T):
        oh_off[t + 1] = oh_off[t] + int(bw[t]) * WIN

    grid = dict(T=T, E_PAD=E_PAD, B_W=B_W, w0=w0, bw=bw, win_tiles=win_tiles,
                oh_off=oh_off)

    import ml_dtypes
    per_core = []
    for k in range(N_CORES):
        per_core.append(dict(
            e_featT=np.ascontiguousarray(ef_pad[k].T).astype(ml_dtypes.bfloat16),  # [10, E_PAD]
            n_featT=np.ascontiguousarray(n_feat[k * NPC:(k + 1) * NPC].T),  # [42, NPC]
            src_idx=np.ascontiguousarray(src_pad[k].reshape(T, 128).T).astype(np.int32),  # [128, T]
            dst_rel=np.ascontiguousarray(dstrel_pad[k].reshape(T, 128).T),  # [128, T]
        ))
    return grid, per_core


def _build_program(grid):
    T, B_W = grid["T"], grid["B_W"]
    w0, bw, win_tiles = grid["w0"], grid["bw"], grid["win_tiles"]
    oh_off = grid["oh_off"]
    OH_TOT = int(oh_off[T])
    PREFIX_MAC = _register_prefix_mac()

    nc = bacc.Bacc("TRN2", target_bir_lowering=False, debug=False,
                   num_devices=N_CORES)

    # ---- kernel I/O ----
    e_featT = nc.dram_tensor("e_featT", [E_IN, grid["E_PAD"]], BF16, kind="ExternalInput")
    n_featT = nc.dram_tensor("n_featT", [D_IN, NPC], F32, kind="ExternalInput")
    src_idx = nc.dram_tensor("src_idx", [128, T], I32, kind="ExternalInput")
    dst_rel = nc.dram_tensor("dst_rel", [128, T], F32, kind="ExternalInput")
    iota = nc.dram_tensor("iota", [128, B_W * WIN], F32, kind="ExternalInput")
    en1_w = nc.dram_tensor("en1_w", [E_IN, E_H], BF16, kind="ExternalInput")
    en1_b = nc.dram_tensor("en1_b", [1, E_H], BF16, kind="ExternalInput")
    en2_wp = nc.dram_tensor("en2_wp", [E_H, NW], BF16, kind="ExternalInput")
    b_r = nc.dram_tensor("b_r", [D_H, D_H], BF16, kind="ExternalInput")
    lin0_wt = nc.dram_tensor("lin0_wt", [D_IN, D_H], F32, kind="ExternalInput")
    lin0_br = nc.dram_tensor("lin0_br", [1, D_H], F32, kind="ExternalInput")
    msgw_top = nc.dram_tensor("msgw_top", [D_H, D_H], F32, kind="ExternalInput")
    msgw_bot = nc.dram_tensor("msgw_bot", [D_H, D_H], F32, kind="ExternalInput")
    msgb_r = nc.dram_tensor("msgb_r", [1, D_H], F32, kind="ExternalInput")
    convb_r = nc.dram_tensor("convb_r", [1, D_H], F32, kind="ExternalInput")
    ident = nc.dram_tensor("ident", [D_H, D_H], F32, kind="ExternalInput")
    ones_r = nc.dram_tensor("ones_r", [1, 128], F32, kind="ExternalInput")
    ones_b = nc.dram_tensor("ones_b", [1, 128], BF16, kind="ExternalInput")
    y = nc.dram_tensor("y", [NPC, D_H], F32, kind="ExternalOutput")

    with tile.TileContext(nc) as tc:
        with (
            tc.tile_pool(name="const", bufs=1) as cpool,
            tc.tile_pool(name="dram", bufs=1, space="DRAM") as dram,
        ):
            # ---- persistent SBUF residents ----
            nfT_sb = cpool.tile([D_IN, NPC], F32)
            srci_sb = cpool.tile([128, T], I32)
            dstr_sb = cpool.tile([128, T], F32)
            iota_sb = cpool.tile([128, B_W * WIN], F32)
            en1w_sb = cpool.tile([E_IN, E_H], BF16)
            en1b_sb = cpool.tile([1, E_H], BF16)
            en2wp_sb = cpool.tile([E_H, NW], BF16)
            br_sb = cpool.tile([D_H, D_H], BF16)
            lin0w_sb = cpool.tile([D_IN, D_H], F32)
            lin0b_sb = cpool.tile([1, D_H], F32)
            mwt_sb = cpool.tile([D_H, D_H], F32)
            mwb_sb = cpool.tile([D_H, D_H], F32)
            mb_sb = cpool.tile([1, D_H], F32)
            cvb_sb = cpool.tile([1, D_H], F32)
            id_sb = cpool.tile([D_H, D_H], F32)
            ones_sb = cpool.tile([1, 128], F32)
            onesb_sb = cpool.tile([1, 128], BF16)
            outT_a = cpool.tile([D_H, NPC], F32)
            outT_b = cpool.tile([D_H, NPC], F32)
            oh_all = cpool.tile([128, OH_TOT], BF16)       # resident one-hots
            pfx = cpool.tile([128, 1 + NW], F32)

            for sb, dr in [(nfT_sb, n_featT), (srci_sb, src_idx),
                           (dstr_sb, dst_rel), (iota_sb, iota), (en1w_sb, en1_w),
                           (en1b_sb, en1_b), (en2wp_sb, en2_wp), (br_sb, b_r),
                           (lin0w_sb, lin0_wt), (lin0b_sb, lin0_br), (mwt_sb, msgw_top),
                           (mwb_sb, msgw_bot), (mb_sb, msgb_r), (cvb_sb, convb_r),
                           (id_sb, ident), (ones_sb, ones_r), (onesb_sb, ones_b)]:
                nc.sync.dma_start(sb[:], dr[:])
            nc.gpsimd.memset(pfx[:, 0:1], 0.0)

            # ---- DRAM scratch ----
            w_dram = dram.tile([T * 128, NW], BF16)
            cc_in = [dram.tile([NPC, D_H], BF16, name=f"cc_in{i}") for i in range(2)]
            cc_out = [dram.tile([N, D_H], BF16, name=f"cc_out{i}", addr_space="Shared")
                      for i in range(STEPS)]

            # =========== setup: build W' (bf16) in HBM; one-hot tiles ===========
            ECH = 16  # e_feat tiles per SBUF chunk
            with (
                tc.tile_pool(name="su_e", bufs=2) as su_e,
                tc.tile_pool(name="su_h", bufs=3) as su_h,
                tc.tile_pool(name="su_w", bufs=3) as su_w,
                tc.tile_pool(name="su_ph", bufs=2, space="PSUM") as su_ph,
                tc.tile_pool(name="su_pw", bufs=3, space="PSUM") as su_pw,
            ):
                e_ch = None
                for t in range(T):
                    if t % ECH == 0:
                        c0 = t * 128
                        c1 = min((t + ECH) * 128, grid["E_PAD"])
                        e_ch = su_e.tile([E_IN, ECH * 128], BF16, name="e_ch")
                        nc.sync.dma_start(e_ch[:, :c1 - c0], e_featT[:, c0:c1])
                    ph = su_ph.tile([128, 128], F32, name="ph")
                    o = (t % ECH) * 128
                    nc.tensor.matmul(ph[:], lhsT=en1w_sb[:], rhs=e_ch[:, o:o + 128],
                                     start=True, stop=False)
                    nc.tensor.matmul(ph[:], lhsT=en1b_sb[:], rhs=onesb_sb[:1, :],
                                     start=False, stop=True)
                    h_sb = su_h.tile([128, 128], BF16, name="h_sb")
                    nc.scalar.activation(h_sb[:], ph[:],
                                         mybir.ActivationFunctionType.Relu)
                    w_sb = su_w.tile([128, NW], BF16, name="w_sb")
                    for hf in range(2):
                        pw = su_pw.tile([128, 2, 512], F32, name="pw")
                        for c in range(2):
                            c0 = (2 * hf + c) * CHUNK
                            nc.tensor.matmul(pw[:, c, :CHUNK], lhsT=h_sb[:],
                                             rhs=en2wp_sb[:, c0:c0 + CHUNK],
                                             start=True, stop=True)
                        # cast fp32 psum -> bf16 sbuf (alternate scalar/vector)
                        dst = w_sb[:, 2 * hf * CHUNK:(2 * hf + 2) * CHUNK]
                        dst3 = dst.rearrange("p (c k) -> p c k", c=2)
                        if hf == 0:
                            nc.scalar.copy(dst3, pw[:, :, :CHUNK])
                        else:
                            nc.vector.tensor_copy(dst3, pw[:, :, :CHUNK])
                    nc.sync.dma_start(w_dram[t * 128:(t + 1) * 128, :], w_sb[:])
                    # one-hot scatter block for this tile (static across steps)
                    bwt = int(bw[t])
                    o0 = int(oh_off[t])
                    nc.vector.tensor_scalar(
                        out=oh_all[:, o0:o0 + bwt * WIN],
                        in0=iota_sb[:, :bwt * WIN],
                        scalar1=dstr_sb[:, t:t + 1],
                        scalar2=None, op0=mybir.AluOpType.is_equal)

            # =========== step pools ===========
            # PSUM: per-window packed pair of banks; "agt" = msg-aggregate +
            # transpose scratch, "ast" = source-feature sum S + update output.
            with (
                tc.tile_pool(name="st_w", bufs=8) as p_w,
                tc.tile_pool(name="st_m", bufs=6) as p_m,
                tc.tile_pool(name="st_sm", bufs=5) as p_sm,
                tc.tile_pool(name="ps_win", bufs=3, space="PSUM") as ps_win,
            ):
                def window_cols(w):
                    n0 = w * WIN
                    m = min(WIN, NPC - n0)
                    return n0, m

                def new_window_tiles():
                    agt = ps_win.tile([128, 512], F32, name="agt")
                    ast = ps_win.tile([D_H, 512], F32, name="ast")
                    return agt, ast

                def update_window(w, outT_cur, outT_new, agt, ast, step):
                    """Window epilogue: S@B bias, residual, relu, update matmul,
                    transpose, DMA rows out."""
                    n0, m = window_cols(w)
                    last = step == STEPS
                    aggr = agt[0:D_H, 0:WIN]
                    s_ps = ast[:, 0:WIN]
                    up = ast[:, WIN:2 * WIN]
                    tr = agt[:, WIN:WIN + D_H]
                    # en2 bias via aggregated source features: aggr += (S @ B)^T
                    s_sb = p_sm.tile([D_H, WIN], BF16, name="s_sb")
                    nc.scalar.copy(s_sb[:, :m], s_ps[:, :m])
                    nc.tensor.matmul(aggr[:, :m], lhsT=br_sb[:],
                                     rhs=s_sb[:, :m], start=False, stop=False)
                    # + out (identity residual into conv) and conv bias
                    nc.tensor.matmul(aggr[:, :m], lhsT=id_sb[:],
                                     rhs=outT_cur[:, n0:n0 + m], start=False, stop=False)
                    nc.tensor.matmul(aggr[:, :m], lhsT=cvb_sb[:],
                                     rhs=ones_sb[:1, :m], start=False, stop=True)
                    mT_sb = p_sm.tile([D_H, WIN], F32, name="mT_sb")
                    nc.scalar.activation(mT_sb[:, :m], aggr[:, :m],
                                         mybir.ActivationFunctionType.Relu)
                    nc.tensor.matmul(up[:, :m], lhsT=mwt_sb[:], rhs=mT_sb[:, :m],
                                     start=True, stop=False)
                    nc.tensor.matmul(up[:, :m], lhsT=mwb_sb[:], rhs=outT_cur[:, n0:n0 + m],
                                     start=False, stop=False)
                    nc.tensor.matmul(up[:, :m], lhsT=mb_sb[:], rhs=ones_sb[:1, :m],
                                     start=False, stop=not last)
                    if last:
                        nc.tensor.matmul(up[:, :m], lhsT=id_sb[:], rhs=nfT_sb[:, n0:n0 + m],
                                         start=False, stop=True)
                    nc.scalar.copy(outT_new[:, n0:n0 + m], up[:, :m])
                    nc.tensor.transpose(tr[:m, :], outT_new[:, n0:n0 + m], id_sb[:])
                    if last:
                        rows = p_sm.tile([128, D_H], F32, name="rows_f")
                        nc.scalar.copy(rows[:m, :], tr[:m, :])
                        nc.sync.dma_start(y[n0:n0 + m, :], rows[:m, :])
                    else:
                        rows = p_sm.tile([128, D_H], BF16, name="rows")
                        nc.scalar.copy(rows[:m, :], tr[:m, :])
                        nc.sync.dma_start(cc_in[step % 2][n0:n0 + m, :], rows[:m, :])

                def all_gather(step):
                    nc.gpsimd.collective_compute(
                        "AllGather", mybir.AluOpType.bypass,
                        replica_groups=[list(range(N_CORES))],
                        ins=[cc_in[step % 2].opt()], outs=[cc_out[step].opt()])

                # =========== lin0: out0 = relu(n_feat @ lin0_w + b) ===========
                for w in range(N_WIN):
                    n0, m = window_cols(w)
                    agt, ast = new_window_tiles()
                    up = ast[:, WIN:2 * WIN]
                    tr = agt[:, WIN:WIN + D_H]
                    nc.tensor.matmul(up[:, :m], lhsT=lin0w_sb[:], rhs=nfT_sb[:, n0:n0 + m],
                                     start=True, stop=False)
                    nc.tensor.matmul(up[:, :m], lhsT=lin0b_sb[:], rhs=ones_sb[:1, :m],
                                     start=False, stop=True)
                    nc.scalar.activation(outT_a[:, n0:n0 + m], up[:, :m],
                                         mybir.ActivationFunctionType.Relu)
                    nc.tensor.transpose(tr[:m, :], outT_a[:, n0:n0 + m], id_sb[:])
                    rows = p_sm.tile([128, D_H], BF16, name="rows")
                    nc.scalar.copy(rows[:m, :], tr[:m, :])
                    nc.sync.dma_start(cc_in[0][n0:n0 + m, :], rows[:m, :])
                all_gather(0)

                # =========== message passing steps ===========
                for step in range(1, STEPS + 1):
                    outT_cur = outT_a if step % 2 == 1 else outT_b
                    outT_new = outT_b if step % 2 == 1 else outT_a
                    src_buf = cc_out[step - 1]
                    aggr_of = {}
                    for t in range(T):
                        # gathered src feats live in cols [42:84) of the
                        # scatter stationary [msg | x]
                        mx = p_m.tile([128, 2 * D_H], BF16, name="mx")
                        nc.gpsimd.indirect_dma_start(
                            out=mx[:, D_H:2 * D_H], out_offset=None, in_=src_buf[:],
                            in_offset=bass.IndirectOffsetOnAxis(
                                ap=srci_sb[:, t:t + 1], axis=0))
                        w_t = p_w.tile([128, NW], BF16, name="w_t")
                        nc.sync.dma_start(w_t[:], w_dram[t * 128:(t + 1) * 128, :])
                        nc.vector._custom_dve(
                            PREFIX_MAC, out=pfx[:, 1:1 + NW], in0=w_t[:],
                            in1=mx[:, D_H:2 * D_H][:, None, :].to_broadcast(
                                [128, D_H, D_H]))
                        nc.vector.tensor_tensor(
                            out=mx[:, 0:D_H],
                            in0=pfx[:, D_H:1 + NW:D_H],
                            in1=pfx[:, 0:NW - D_H + 1:D_H],
                            op=mybir.AluOpType.subtract)
                        # scatter matmuls: message aggregate + source-feat sum S
                        bwt = int(bw[t])
                        o0 = int(oh_off[t])
                        for j in range(bwt):
                            w = int(w0[t]) + j
                            if w >= N_WIN:
                                continue
                            tiles_w = win_tiles[w]
                            if w not in aggr_of:
                                aggr_of[w] = new_window_tiles()
                            first = t == tiles_w[0]
                            last_t = t == tiles_w[-1]
                            oh_j = oh_all[:, o0 + j * WIN:o0 + (j + 1) * WIN]
                            agt, ast = aggr_of[w]
                            nc.tensor.matmul(agt[0:D_H, 0:WIN], lhsT=mx[:, 0:D_H],
                                             rhs=oh_j, start=first, stop=False)
                            nc.tensor.matmul(ast[:, 0:WIN], lhsT=mx[:, D_H:2 * D_H],
                                             rhs=oh_j, start=first, stop=last_t)
                            if last_t:
                                update_window(w, outT_cur, outT_new,
                                              agt, ast, step)
                                aggr_of.pop(w)
                    if step < STEPS:
                        all_gather(step)

    nc.compile()
    return nc


_CACHED = {}


def kernel(n_feat, e_feat, src, dst, lin0_w, lin0_b, en1_w, en1_b,
           en2_w, en2_b, conv_bias, msg_w, msg_b):
    import ml_dtypes
    n_feat = np.asarray(n_feat, dtype=np.float32)
    e_feat = np.asarray(e_feat, dtype=np.float32)
    src = np.asarray(src, dtype=np.int32)
    dst = np.asarray(dst, dtype=np.int32)

    grid, per_core = _host_prep(n_feat, e_feat, src, dst)

    key = (grid["T"], grid["B_W"], tuple(grid["w0"].tolist()))
    if key not in _CACHED:
        _CACHED.clear()
        _CACHED[key] = _build_program(grid)
    nc = _CACHED[key]

    # en2_w reshaped so W' columns are (o, i) o-major, matching the scan's
    # per-o prefix-difference extraction
    en2_wp = np.ascontiguousarray(
        np.asarray(en2_w, np.float32).reshape(E_H, D_H, D_H).transpose(0, 2, 1)
        .reshape(E_H, NW)).astype(ml_dtypes.bfloat16)
    shared = dict(
        iota=np.tile(np.arange(grid["B_W"] * WIN, dtype=np.float32), (128, 1)),
        en1_w=np.asarray(en1_w, np.float32).astype(ml_dtypes.bfloat16),
        en1_b=np.asarray(en1_b, np.float32).reshape(1, E_H).astype(ml_dtypes.bfloat16),
        en2_wp=en2_wp,
        b_r=np.ascontiguousarray(
            np.asarray(en2_b, np.float32).reshape(D_H, D_H)).astype(ml_dtypes.bfloat16),
        lin0_wt=np.asarray(lin0_w, np.float32),
        lin0_br=np.asarray(lin0_b, np.float32).reshape(1, D_H),
        msgw_top=np.ascontiguousarray(np.asarray(msg_w, np.float32)[:D_H, :]),
        msgw_bot=np.ascontiguousarray(np.asarray(msg_w, np.float32)[D_H:, :]),
        msgb_r=np.asarray(msg_b, np.float32).reshape(1, D_H),
        convb_r=np.asarray(conv_bias, np.float32).reshape(1, D_H),
        ident=np.eye(D_H, dtype=np.float32),
        ones_r=np.ones((1, 128), dtype=np.float32),
        ones_b=np.ones((1, 128), dtype=ml_dtypes.bfloat16),
    )
    in_maps = []
    for k in range(N_CORES):
        m = dict(shared)
        m.update(per_core[k])
        in_maps.append(m)

    res = bass_utils.run_bass_kernel_spmd(nc, in_maps, core_ids=list(range(N_CORES)))
    out = np.concatenate([res.results[k]["y"] for k in range(N_CORES)], axis=0)
    return out.astype(np.float32)


# revision 10
# speedup vs baseline: 1.5310x; 1.5310x over previous
"""Trainium2 Bass kernel for nn_GatherModel (NNConv GNN message passing).

8-core SPMD, edge-parallel sharded by destination node block.

v4 design:
  - Node ids are globally permuted on the host so every (core, 128-node
    window) bin has a near-equal number of incoming edges (LPT packing).
    This balances the 8 cores and shrinks the shared edge-tile grid from
    158 to ~147 tiles/core.
  - Per-edge weights W' = h_e @ en2_w' (bf16) are handled half-and-half:
    o-blocks 21..41 are built once in setup and streamed from HBM
    (226 KB/tile), while o-blocks 0..20 are rebuilt each step on the
    Tensor engine straight into PSUM from the SBUF-resident h (bf16).
    This splits the load between the DMA queues and the PE so neither
    outruns the Vector engine's multiply+prefix-scan contraction, which
    is the critical path (~2.3 us/tile).
  - The en2 bias never enters W': the scatter matmul aggregates both
    messages and raw source features (S), and each window epilogue adds
    S @ B with one small matmul (bias-via-scatter, exact by linearity).
  - scatter (segment-sum over dst) is a PE matmul against precomputed
    one-hot window matrices (SBUF-resident bf16, built once in setup)
  - node update runs in fp32 transposed feature layout; updated features
    are cast to bf16 rows and AllGather'd across the 8 cores each step.
"""
import heapq

import numpy as np

import concourse.bacc as bacc
import concourse.bass as bass
import concourse.mybir as mybir
import concourse.tile as tile
from concourse import bass_utils, dve_ops
from concourse.dve_spec import Spec, Src0, Src1, scan, AluOp, lower, _has_src1
from concourse.dve_uop import DveOpSpec

N = 50000
E = 150000
D_IN = 42
D_H = 42
E_IN = 10
E_H = 128
STEPS = 6
N_CORES = 8
NPC = N // N_CORES          # 6250 nodes per core
WIN = 128                   # scatter window (node block) size
N_WIN = (NPC + WIN - 1) // WIN  # 49 windows per core, last partial (106)
NW = D_H * D_H              # 1764
HNW = NW // 2               # 882 = 21 o-blocks of 42
O_HALF = D_H // 2           # 21
CHUNK = 441                 # W-build matmul chunk (1 PSUM bank holds 512 fp32)
F32 = mybir.dt.float32
BF16 = mybir.dt.bfloat16
I32 = mybir.dt.int32


def _register_prefix_mac():
    name = "PREFIX_MAC_GNN"
    if name in dve_ops._SUB_OPCODE_FOR_NAME:
        return next(op for op in dve_ops.OPS if op.name == name)
    spec = Spec(
        body=scan(AluOp.ADD, Src0 * Src1),
        reference=lambda in0, in1, s0, s1, imm2: np.cumsum(
            (in0.astype(np.float32) * in1).reshape(in0.shape[0], -1), axis=-1
        ),
    )
    shas = {}
    row = dve_ops._CUSTOM_DVE_ROW_BASE + len(dve_ops.OPS)
    for ver in ("v3", "v4"):
        uops = lower(spec, ver=ver)
        shas[ver] = DveOpSpec(name=name, opcode=row, uops=uops,
                              rd1_en=_has_src1(spec)).sha(ver)
    op = dve_ops.DveOp(name, spec, subdim=False, uops_sha=shas)
    dve_ops.OPS.append(op)
    dve_ops._SUB_OPCODE_FOR_NAME[name] = row
    dve_ops.CUSTOM_DVE_SPECS[name] = spec
    return op


def _balance_nodes(dst):
    """LPT-pack nodes into (core, local-window) bins to equalize per-bin
    in-degree sums; returns (old->new, new->old) node id permutations."""
    deg = np.bincount(dst, minlength=N).astype(np.int64)
    caps = np.full(N_CORES * N_WIN, WIN, dtype=np.int64)
    caps[N_WIN - 1::N_WIN] = NPC - (N_WIN - 1) * WIN
    order = np.argsort(-deg, kind="stable")
    heap = [(0, b) for b in range(N_CORES * N_WIN)]
    heapq.heapify(heap)
    fill = np.zeros(N_CORES * N_WIN, dtype=np.int64)
    sums = np.zeros(N_CORES * N_WIN, dtype=np.int64)
    members = [[] for _ in range(N_CORES * N_WIN)]
    for v in order:
        while True:
            s, b = heapq.heappop(heap)
            if fill[b] < caps[b]:
                break
        members[b].append(v)
        fill[b] += 1
        sums[b] += deg[v]
        if fill[b] < caps[b]:
            heapq.heappush(heap, (sums[b], b))
    new2old = np.empty(N, dtype=np.int64)
    pos = 0
    for b in range(N_CORES * N_WIN):
        m = members[b]
        new2old[pos:pos + len(m)] = m
        pos += len(m)
    old2new = np.empty(N, dtype=np.int64)
    old2new[new2old] = np.arange(N)
    return old2new, new2old


def _host_prep(n_feat, e_feat, src, dst):
    """Sort edges by dst, shard by dst block, pad each (core, window) edge run
    onto a shared slot grid so the tile->window map is identical on all cores."""
    order = np.argsort(dst, kind="stable")
    src_s, dst_s, ef_s = src[order], dst[order], e_feat[order]

    # per (core, window) counts
    core_e = dst_s // NPC
    loc = dst_s - core_e * NPC
    win_e = loc // WIN
    cnt = np.zeros((N_CORES, N_WIN), dtype=np.int64)
    np.add.at(cnt, (core_e, win_e), 1)

    slot_cnt = cnt.max(axis=0)                       # shared grid
    G = np.concatenate([[0], np.cumsum(slot_cnt)])   # window slot boundaries
    total = int(G[-1])
    T = (total + 127) // 128                         # edge tiles per core
    E_PAD = T * 128

    # per-core padded edge arrays
    src_pad = np.zeros((N_CORES, E_PAD), dtype=np.int32)
    dstrel_pad = np.full((N_CORES, E_PAD), -1.0, dtype=np.float32)
    ef_pad = np.zeros((N_CORES, E_PAD, E_IN), dtype=np.float32)

    # tile -> window band
    w0 = np.zeros(T, dtype=np.int64)       # first window overlapping tile t
    bw = np.zeros(T, dtype=np.int64)       # how many windows overlap tile t
    for t in range(T):
        lo, hi = t * 128, min((t + 1) * 128, total)
        wlo = int(np.searchsorted(G, lo, side="right") - 1)
        whi = int(np.searchsorted(G, max(hi - 1, lo), side="right") - 1)
        wlo, whi = min(wlo, N_WIN - 1), min(whi, N_WIN - 1)
        w0[t] = wlo
        bw[t] = whi - wlo + 1
    B_W = int(bw.max())

    # fill padded arrays: window w of core k occupies slots [G[w], G[w]+cnt[k,w])
    core_starts = np.searchsorted(core_e, np.arange(N_CORES))
    for k in range(N_CORES):
        base = core_starts[k]
        cw = np.concatenate([[0], np.cumsum(cnt[k])])
        for w in range(N_WIN):
            s0, s1 = int(base + cw[w]), int(base + cw[w + 1])
            g0 = int(G[w])
            n_e = s1 - s0
            src_pad[k, g0:g0 + n_e] = src_s[s0:s1]
            ef_pad[k, g0:g0 + n_e] = ef_s[s0:s1]
            # dst_rel relative to the band anchor of the edge's tile
            slots = np.arange(g0, g0 + n_e)
            dstrel_pad[k, g0:g0 + n_e] = (
                loc[s0:s1] - w0[slots // 128] * WIN).astype(np.float32)

    # scatter pair list (t, w) from actual overlap, and per-window tile ranges
    pairs = []
    for t in range(T):
        for j in range(int(bw[t])):
            w = int(w0[t]) + j
            if w < N_WIN:
                pairs.append((t, w))
    win_tiles = {w: [t for (t, ww) in pairs if ww == w] for w in range(N_WIN)}

    # offset of each tile's one-hot block inside the resident oh buffer
    oh_off = np.zeros(T + 1, dtype=np.int64)
    for t in range(T):
        oh_off[t + 1] = oh_off[t] + int(bw[t]) * WIN

    grid = dict(T=T, E_PAD=E_PAD, B_W=B_W, w0=w0, bw=bw, win_tiles=win_tiles,
                oh_off=oh_off)

    import ml_dtypes
    per_core = []
    for k in range(N_CORES):
        per_core.append(dict(
            e_featT=np.ascontiguousarray(ef_pad[k].T).astype(ml_dtypes.bfloat16),  # [10, E_PAD]
            n_featT=np.ascontiguousarray(n_feat[k * NPC:(k + 1) * NPC].T),  # [42, NPC]
            src_idx=np.ascontiguousarray(src_pad[k].reshape(T, 128).T).astype(np.int32),  # [128, T]
            dst_rel=np.ascontiguousarray(dstrel_pad[k].reshape(T, 128).T),  # [128, T]
        ))
    return grid, per_core


def _build_program(grid):
    T, B_W = grid["T"], grid["B_W"]
    w0, bw, win_tiles = grid["w0"], grid["bw"], grid["win_tiles"]
    oh_off = grid["oh_off"]
    OH_TOT = int(oh_off[T])
    PREFIX_MAC = _register_prefix_mac()

    nc = bacc.Bacc("TRN2", target_bir_lowering=False, debug=False,
                   num_devices=N_CORES)

    # ---- kernel I/O ----
    e_featT = nc.dram_tensor("e_featT", [E_IN, grid["E_PAD"]], BF16, kind="ExternalInput")
    n_featT = nc.dram_tensor("n_featT", [D_IN, NPC], F32, kind="ExternalInput")
    src_idx = nc.dram_tensor("src_idx", [128, T], I32, kind="ExternalInput")
    dst_rel = nc.dram_tensor("dst_rel", [128, T], F32, kind="ExternalInput")
    iota = nc.dram_tensor("iota", [128, B_W * WIN], F32, kind="ExternalInput")
    en1_w = nc.dram_tensor("en1_w", [E_IN, E_H], BF16, kind="ExternalInput")
    en1_b = nc.dram_tensor("en1_b", [1, E_H], BF16, kind="ExternalInput")
    en2_wp = nc.dram_tensor("en2_wp", [E_H, NW], BF16, kind="ExternalInput")
    b_r = nc.dram_tensor("b_r", [D_H, D_H], BF16, kind="ExternalInput")
    lin0_wt = nc.dram_tensor("lin0_wt", [D_IN, D_H], F32, kind="ExternalInput")
    lin0_br = nc.dram_tensor("lin0_br", [1, D_H], F32, kind="ExternalInput")
    msgw_top = nc.dram_tensor("msgw_top", [D_H, D_H], F32, kind="ExternalInput")
    msgw_bot = nc.dram_tensor("msgw_bot", [D_H, D_H], F32, kind="ExternalInput")
    msgb_r = nc.dram_tensor("msgb_r", [1, D_H], F32, kind="ExternalInput")
    convb_r = nc.dram_tensor("convb_r", [1, D_H], F32, kind="ExternalInput")
    ident = nc.dram_tensor("ident", [D_H, D_H], F32, kind="ExternalInput")
    ones_r = nc.dram_tensor("ones_r", [1, 128], F32, kind="ExternalInput")
    ones_b = nc.dram_tensor("ones_b", [1, 128], BF16, kind="ExternalInput")
    y = nc.dram_tensor("y", [NPC, D_H], F32, kind="ExternalOutput")

    with tile.TileContext(nc) as tc:
        with (
            tc.tile_pool(name="const", bufs=1) as cpool,
            tc.tile_pool(name="dram", bufs=1, space="DRAM") as dram,
        ):
            # ---- persistent SBUF residents ----
            nfT_sb = cpool.tile([D_IN, NPC], F32)
            srci_sb = cpool.tile([128, T], I32)
            dstr_sb = cpool.tile([128, T], F32)
            iota_sb = cpool.tile([128, B_W * WIN], F32)
            en1w_sb = cpool.tile([E_IN, E_H], BF16)
            en1b_sb = cpool.tile([1, E_H], BF16)
            en2wp_sb = cpool.tile([E_H, NW], BF16)
            br_sb = cpool.tile([D_H, D_H], BF16)
            lin0w_sb = cpool.tile([D_IN, D_H], F32)
            lin0b_sb = cpool.tile([1, D_H], F32)
            mwt_sb = cpool.tile([D_H, D_H], F32)
            mwb_sb = cpool.tile([D_H, D_H], F32)
            mb_sb = cpool.tile([1, D_H], F32)
            cvb_sb = cpool.tile([1, D_H], F32)
            id_sb = cpool.tile([D_H, D_H], F32)
            ones_sb = cpool.tile([1, 128], F32)
            onesb_sb = cpool.tile([1, 128], BF16)
            outT_a = cpool.tile([D_H, NPC], F32)
            outT_b = cpool.tile([D_H, NPC], F32)
            h_all = cpool.tile([128, T * 128], BF16)       # resident h^T tiles
            oh_all = cpool.tile([128, OH_TOT], BF16)       # resident one-hots
            # two prefix-scan halves, each: zero cell + 882 sums + pad
            pfx = cpool.tile([128, 2, HNW + 2], F32)

            for sb, dr in [(nfT_sb, n_featT), (srci_sb, src_idx),
                           (dstr_sb, dst_rel), (iota_sb, iota), (en1w_sb, en1_w),
                           (en1b_sb, en1_b), (en2wp_sb, en2_wp), (br_sb, b_r),
                           (lin0w_sb, lin0_wt), (lin0b_sb, lin0_br), (mwt_sb, msgw_top),
                           (mwb_sb, msgw_bot), (mb_sb, msgb_r), (cvb_sb, convb_r),
                           (id_sb, ident), (ones_sb, ones_r), (onesb_sb, ones_b)]:
                nc.sync.dma_start(sb[:], dr[:])
            nc.gpsimd.memset(pfx[:, :, 0:1], 0.0)

            # ---- DRAM scratch ----
            w_dram = dram.tile([T * 128, HNW], BF16)       # streamed W half
            cc_in = [dram.tile([NPC, D_H], BF16, name=f"cc_in{i}") for i in range(2)]
            cc_out = [dram.tile([N, D_H], BF16, name=f"cc_out{i}", addr_space="Shared")
                      for i in range(STEPS)]

            # ==== setup: h tiles, streamed W half (o-blocks 21..41), one-hots
            ECH = 16  # e_feat tiles per SBUF chunk
            with (
                tc.tile_pool(name="su_e", bufs=2) as su_e,
                tc.tile_pool(name="su_w", bufs=3) as su_w,
                tc.tile_pool(name="su_ph", bufs=2, space="PSUM") as su_ph,
                tc.tile_pool(name="su_pw", bufs=3, space="PSUM") as su_pw,
            ):
                e_ch = None
                for t in range(T):
                    if t % ECH == 0:
                        c0 = t * 128
                        c1 = min((t + ECH) * 128, grid["E_PAD"])
                        e_ch = su_e.tile([E_IN, ECH * 128], BF16, name="e_ch")
                        nc.sync.dma_start(e_ch[:, :c1 - c0], e_featT[:, c0:c1])
                    ph = su_ph.tile([128, 128], F32, name="ph")
                    o = (t % ECH) * 128
                    nc.tensor.matmul(ph[:], lhsT=en1w_sb[:], rhs=e_ch[:, o:o + 128],
                                     start=True, stop=False)
                    nc.tensor.matmul(ph[:], lhsT=en1b_sb[:], rhs=onesb_sb[:1, :],
                                     start=False, stop=True)
                    h_t = h_all[:, t * 128:(t + 1) * 128]
                    nc.scalar.activation(h_t, ph[:],
                                         mybir.ActivationFunctionType.Relu)
                    pw = su_pw.tile([128, 2, 512], F32, name="pw")
                    w_sb = su_w.tile([128, HNW], BF16, name="w_sb")
                    for c in range(2):
                        c0 = (2 + c) * CHUNK
                        nc.tensor.matmul(pw[:, c, :CHUNK], lhsT=h_t,
                                         rhs=en2wp_sb[:, c0:c0 + CHUNK],
                                         start=True, stop=True)
                    # cast fp32 psum -> bf16 sbuf (alternate scalar/vector)
                    dst3 = w_sb[:].rearrange("p (c k) -> p c k", c=2)
                    if t % 2 == 0:
                        nc.scalar.copy(dst3, pw[:, :, :CHUNK])
                    else:
                        nc.vector.tensor_copy(dst3, pw[:, :, :CHUNK])
                    nc.sync.dma_start(w_dram[t * 128:(t + 1) * 128, :], w_sb[:])
                    # one-hot scatter block for this tile (static across steps)
                    bwt = int(bw[t])
                    o0 = int(oh_off[t])
                    nc.gpsimd.tensor_scalar(
                        out=oh_all[:, o0:o0 + bwt * WIN],
                        in0=iota_sb[:, :bwt * WIN],
                        scalar1=dstr_sb[:, t:t + 1],
                        scalar2=None, op0=mybir.AluOpType.is_equal)

            # =========== step pools ===========
            # PSUM (8 banks): rebuilt-W pipeline 2x2; per-window packed pair
            # 2x(1+1): "agt" = msg-aggregate + transpose scratch, "ast" =
            # source-feature sum S + update output.
            with (
                tc.tile_pool(name="st_w", bufs=8) as p_w,
                tc.tile_pool(name="st_m", bufs=8) as p_m,
                tc.tile_pool(name="st_sm", bufs=5) as p_sm,
                tc.tile_pool(name="ps_w", bufs=2, space="PSUM") as ps_w,
                tc.tile_pool(name="ps_win", bufs=2, space="PSUM") as ps_win,
            ):
                def window_cols(w):
                    n0 = w * WIN
                    m = min(WIN, NPC - n0)
                    return n0, m

                def new_window_tiles():
                    agt = ps_win.tile([128, 512], F32, name="agt")
                    ast = ps_win.tile([D_H, 512], F32, name="ast")
                    return agt, ast

                def update_window(w, outT_cur, outT_new, agt, ast, step):
                    """Window epilogue: S@B bias, residual, relu, update matmul,
                    transpose, DMA rows out."""
                    n0, m = window_cols(w)
                    last = step == STEPS
                    aggr = agt[0:D_H, 0:WIN]
                    s_ps = ast[:, 0:WIN]
                    up = ast[:, WIN:2 * WIN]
                    tr = agt[:, WIN:WIN + D_H]
                    # en2 bias via aggregated source features: aggr += (S @ B)^T
                    s_sb = p_sm.tile([D_H, WIN], BF16, name="s_sb")
                    nc.scalar.copy(s_sb[:, :m], s_ps[:, :m])
                    nc.tensor.matmul(aggr[:, :m], lhsT=br_sb[:],
                                     rhs=s_sb[:, :m], start=False, stop=False)
                    # + out (identity residual into conv) and conv bias
                    nc.tensor.matmul(aggr[:, :m], lhsT=id_sb[:],
                                     rhs=outT_cur[:, n0:n0 + m], start=False, stop=False)
                    nc.tensor.matmul(aggr[:, :m], lhsT=cvb_sb[:],
                                     rhs=ones_sb[:1, :m], start=False, stop=True)
                    mT_sb = p_sm.tile([D_H, WIN], F32, name="mT_sb")
                    nc.scalar.activation(mT_sb[:, :m], aggr[:, :m],
                                         mybir.ActivationFunctionType.Relu)
                    nc.tensor.matmul(up[:, :m], lhsT=mwt_sb[:], rhs=mT_sb[:, :m],
                                     start=True, stop=False)
                    nc.tensor.matmul(up[:, :m], lhsT=mwb_sb[:], rhs=outT_cur[:, n0:n0 + m],
                                     start=False, stop=False)
                    nc.tensor.matmul(up[:, :m], lhsT=mb_sb[:], rhs=ones_sb[:1, :m],
                                     start=False, stop=not last)
                    if last:
                        nc.tensor.matmul(up[:, :m], lhsT=id_sb[:], rhs=nfT_sb[:, n0:n0 + m],
                                         start=False, stop=True)
                    nc.scalar.copy(outT_new[:, n0:n0 + m], up[:, :m])
                    nc.tensor.transpose(tr[:m, :], outT_new[:, n0:n0 + m], id_sb[:])
                    if last:
                        rows = p_sm.tile([128, D_H], F32, name="rows_f")
                        nc.scalar.copy(rows[:m, :], tr[:m, :])
                        nc.sync.dma_start(y[n0:n0 + m, :], rows[:m, :])
                    else:
                        rows = p_sm.tile([128, D_H], BF16, name="rows")
                        nc.scalar.copy(rows[:m, :], tr[:m, :])
                        nc.sync.dma_start(cc_in[step % 2][n0:n0 + m, :], rows[:m, :])

                def all_gather(step):
                    nc.gpsimd.collective_compute(
                        "AllGather", mybir.AluOpType.bypass,
                        replica_groups=[list(range(N_CORES))],
                        ins=[cc_in[step % 2].opt()], outs=[cc_out[step].opt()])

                # =========== lin0: out0 = relu(n_feat @ lin0_w + b) ===========
                for w in range(N_WIN):
                    n0, m = window_cols(w)
                    agt, ast = new_window_tiles()
                    up = ast[:, WIN:2 * WIN]
                    tr = agt[:, WIN:WIN + D_H]
                    nc.tensor.matmul(up[:, :m], lhsT=lin0w_sb[:], rhs=nfT_sb[:, n0:n0 + m],
                                     start=True, stop=False)
                    nc.tensor.matmul(up[:, :m], lhsT=lin0b_sb[:], rhs=ones_sb[:1, :m],
                                     start=False, stop=True)
                    nc.scalar.activation(outT_a[:, n0:n0 + m], up[:, :m],
                                         mybir.ActivationFunctionType.Relu)
                    nc.tensor.transpose(tr[:m, :], outT_a[:, n0:n0 + m], id_sb[:])
                    rows = p_sm.tile([128, D_H], BF16, name="rows")
                    nc.scalar.copy(rows[:m, :], tr[:m, :])
                    nc.sync.dma_start(cc_in[0][n0:n0 + m, :], rows[:m, :])
                all_gather(0)

                # =========== message passing steps ===========
                for step in range(1, STEPS + 1):
                    outT_cur = outT_a if step % 2 == 1 else outT_b
                    outT_new = outT_b if step % 2 == 1 else outT_a
                    src_buf = cc_out[step - 1]
                    aggr_of = {}
                    for t in range(T):
                        h_t = h_all[:, t * 128:(t + 1) * 128]
                        # gathered src feats live in cols [42:84) of the
                        # scatter stationary [msg | x]
                        mx = p_m.tile([128, 2 * D_H], BF16, name="mx")
                        nc.gpsimd.indirect_dma_start(
                            out=mx[:, D_H:2 * D_H], out_offset=None, in_=src_buf[:],
                            in_offset=bass.IndirectOffsetOnAxis(
                                ap=srci_sb[:, t:t + 1], axis=0))
                        x_bc = mx[:, D_H:2 * D_H][:, None, :].to_broadcast(
                            [128, O_HALF, D_H])
                        # half a (o-blocks 0..20): rebuild on PE into PSUM
                        wp = ps_w.tile([128, 2, 512], F32, name="wp")
                        for c in range(2):
                            nc.tensor.matmul(wp[:, c, :CHUNK], lhsT=h_t,
                                             rhs=en2wp_sb[:, c * CHUNK:(c + 1) * CHUNK],
                                             start=True, stop=True)
                        nc.vector._custom_dve(
                            PREFIX_MAC, out=pfx[:, 0, 1:HNW + 1],
                            in0=wp[:, :, :CHUNK], in1=x_bc)
                        # half b (o-blocks 21..41): streamed from HBM
                        w_t = p_w.tile([128, HNW], BF16, name="w_t")
                        nc.sync.dma_start(w_t[:], w_dram[t * 128:(t + 1) * 128, :])
                        nc.vector._custom_dve(
                            PREFIX_MAC, out=pfx[:, 1, 1:HNW + 1],
                            in0=w_t[:], in1=x_bc)
                        nc.vector.tensor_tensor(
                            out=mx[:, 0:D_H].rearrange("p (h o) -> p h o", h=2),
                            in0=pfx[:, :, D_H:HNW + 1:D_H],
                            in1=pfx[:, :, 0:HNW - D_H + 1:D_H],
                            op=mybir.AluOpType.subtract)
                        # scatter matmuls: message aggregate + source-feat sum S
                        bwt = int(bw[t])
                        o0 = int(oh_off[t])
                        for j in range(bwt):
                            w = int(w0[t]) + j
                            if w >= N_WIN:
                                continue
                            tiles_w = win_tiles[w]
                            if w not in aggr_of:
                                aggr_of[w] = new_window_tiles()
                            first = t == tiles_w[0]
                            last_t = t == tiles_w[-1]
                            oh_j = oh_all[:, o0 + j * WIN:o0 + (j + 1) * WIN]
                            agt, ast = aggr_of[w]
                            nc.tensor.matmul(agt[0:D_H, 0:WIN], lhsT=mx[:, 0:D_H],
                                             rhs=oh_j, start=first, stop=False)
                            nc.tensor.matmul(ast[:, 0:WIN], lhsT=mx[:, D_H:2 * D_H],
                                             rhs=oh_j, start=first, stop=last_t)
                            if last_t:
                                update_window(w, outT_cur, outT_new,
                                              agt, ast, step)
                                aggr_of.pop(w)
                    if step < STEPS:
                        all_gather(step)

    nc.compile()
    return nc


_CACHED = {}


def kernel(n_feat, e_feat, src, dst, lin0_w, lin0_b, en1_w, en1_b,
           en2_w, en2_b, conv_bias, msg_w, msg_b):
    import ml_dtypes
    n_feat = np.asarray(n_feat, dtype=np.float32)
    e_feat = np.asarray(e_feat, dtype=np.float32)
    src = np.asarray(src, dtype=np.int32)
    dst = np.asarray(dst, dtype=np.int32)

    # balance (core, window) loads via a global node relabeling
    old2new, new2old = _balance_nodes(dst)
    n_feat_p = np.ascontiguousarray(n_feat[new2old])
    src_p = old2new[src].astype(np.int32)
    dst_p = old2new[dst].astype(np.int32)

    grid, per_core = _host_prep(n_feat_p, e_feat, src_p, dst_p)

    key = (grid["T"], grid["B_W"], tuple(grid["w0"].tolist()))
    if key not in _CACHED:
        _CACHED.clear()
        _CACHED[key] = _build_program(grid)
    nc = _CACHED[key]

    # en2_w reshaped so W' columns are (o, i) o-major, matching the scan's
    # per-o prefix-difference extraction
    en2_wp = np.ascontiguousarray(
        np.asarray(en2_w, np.float32).reshape(E_H, D_H, D_H).transpose(0, 2, 1)
        .reshape(E_H, NW)).astype(ml_dtypes.bfloat16)
    shared = dict(
        iota=np.tile(np.arange(grid["B_W"] * WIN, dtype=np.float32), (128, 1)),
        en1_w=np.asarray(en1_w, np.float32).astype(ml_dtypes.bfloat16),
        en1_b=np.asarray(en1_b, np.float32).reshape(1, E_H).astype(ml_dtypes.bfloat16),
        en2_wp=en2_wp,
        b_r=np.ascontiguousarray(
            np.asarray(en2_b, np.float32).reshape(D_H, D_H)).astype(ml_dtypes.bfloat16),
        lin0_wt=np.asarray(lin0_w, np.float32),
        lin0_br=np.asarray(lin0_b, np.float32).reshape(1, D_H),
        msgw_top=np.ascontiguousarray(np.asarray(msg_w, np.float32)[:D_H, :]),
        msgw_bot=np.ascontiguousarray(np.asarray(msg_w, np.float32)[D_H:, :]),
        msgb_r=np.asarray(msg_b, np.float32).reshape(1, D_H),
        convb_r=np.asarray(conv_bias, np.float32).reshape(1, D_H),
        ident=np.eye(D_H, dtype=np.float32),
        ones_r=np.ones((1, 128), dtype=np.float32),
        ones_b=np.ones((1, 128), dtype=ml_dtypes.bfloat16),
    )
    in_maps = []
    for k in range(N_CORES):
        m = dict(shared)
        m.update(per_core[k])
        in_maps.append(m)

    res = bass_utils.run_bass_kernel_spmd(nc, in_maps, core_ids=list(range(N_CORES)))
    out_p = np.concatenate([res.results[k]["y"] for k in range(N_CORES)], axis=0)
    out = np.empty_like(out_p)
    out[new2old] = out_p                      # undo the node relabeling
    return out.astype(np.float32)


# revision 13
# speedup vs baseline: 1.5327x; 1.0011x over previous
"""Trainium2 Bass kernel for nn_GatherModel (NNConv GNN message passing).

8-core SPMD, edge-parallel sharded by destination node block.

v4 design:
  - Node ids are globally permuted on the host so every (core, 128-node
    window) bin has a near-equal number of incoming edges (LPT packing).
    This balances the 8 cores and shrinks the shared edge-tile grid from
    158 to ~147 tiles/core.
  - Per-edge weights W' = h_e @ en2_w' (bf16) are handled half-and-half:
    o-blocks 21..41 are built once in setup and streamed from HBM
    (226 KB/tile), while o-blocks 0..20 are rebuilt each step on the
    Tensor engine straight into PSUM from the SBUF-resident h (bf16).
    This splits the load between the DMA queues and the PE so neither
    outruns the Vector engine's multiply+prefix-scan contraction, which
    is the critical path (~2.3 us/tile).
  - The en2 bias never enters W': the scatter matmul aggregates both
    messages and raw source features (S), and each window epilogue adds
    S @ B with one small matmul (bias-via-scatter, exact by linearity).
  - scatter (segment-sum over dst) is a PE matmul against precomputed
    one-hot window matrices (SBUF-resident bf16, built once in setup)
  - node update runs in fp32 transposed feature layout; updated features
    are cast to bf16 rows and AllGather'd across the 8 cores each step.
"""
import heapq

import numpy as np

import concourse.bacc as bacc
import concourse.bass as bass
import concourse.mybir as mybir
import concourse.tile as tile
from concourse import bass_utils, dve_ops
from concourse.dve_spec import Spec, Src0, Src1, scan, AluOp, lower, _has_src1
from concourse.dve_uop import (
    DveOpSpec,
    UopConfig,
    UopDpConfig,
    AluInp,
    DelayInp,
    InpSel,
    OutPath,
    OutSel,
    Trigger,
)
from concourse.dve_uop import AluOp as UAluOp

N = 50000
E = 150000
D_IN = 42
D_H = 42
E_IN = 10
E_H = 128
STEPS = 6
N_CORES = 8
NPC = N // N_CORES          # 6250 nodes per core
WIN = 128                   # scatter window (node block) size
N_WIN = (NPC + WIN - 1) // WIN  # 49 windows per core, last partial (106)
NW = D_H * D_H              # 1764
HNW = NW // 2               # 882 = 21 o-blocks of 42
O_HALF = D_H // 2           # 21
CHUNK = 441                 # W-build matmul chunk (1 PSUM bank holds 512 fp32)
F32 = mybir.dt.float32
BF16 = mybir.dt.bfloat16
I32 = mybir.dt.int32


def _prefix_mac_2x_uops():
    """Hand-built 2X_1PORT program for scan(ADD, Src0*Src1): per cycle the
    engine delivers a (lo, hi) pair per source; we compute both products,
    fold their sum into the running prefix with the same one-cycle ALU
    feedback as the 1x program, and emit lo = p - b via a delay-chained
    subtract while hi rides delay chain 2 to the write mux. Mirrors the
    stock tensor_mask 2x row's conventions (extra SRC_*_HI input lanes,
    write0 hi+lo enables)."""
    # enabled lanes (order -> delay chains): SRC_0, SRC_1, ZERO, SRC_0_HI,
    # SRC_1_HI -> chains 0..4; same lane-packing the lowered 1x uses.
    def wire_inputs(u):
        u.enable_input(InpSel.SRC_0, 1)
        u.enable_input(InpSel.SRC_1, 2)
        u.enable_input(InpSel.ZERO, 3)
        u.enable_input(InpSel.SRC_0_HI, 4)
        u.enable_input(InpSel.SRC_1_HI, 5)

    seed = UopConfig()
    wire_inputs(seed)
    b = seed.datapath_config
    b[0].enable_alu(UAluOp.MULTIPLY, AluInp.PREV_DELAY_0, AluInp.PREV_DELAY_1)
    b[0].pass_through_delay(2, 3, 4)
    b[1].enable_alu(UAluOp.MULTIPLY, AluInp.PREV_DELAY_3, AluInp.PREV_DELAY_4)
    b[1].pass_through_delay(2)
    b[2].pass_through_alu()
    b[2].pass_through_delay(2)
    b[3].enable_alu(UAluOp.BYPASS, AluInp.PREV_DELAY_2)   # accumulator <- 0
    for k in range(4, 8):
        b[k].pass_through_alu()
    seed.repeat_count = 1
    seed.trigger = (Trigger.COUNT, Trigger.NONE, Trigger.NONE)
    seed.next_uop = (1, 0, 0)

    st = UopConfig()
    wire_inputs(st)
    d = st.datapath_config
    d[0].enable_alu(UAluOp.MULTIPLY, AluInp.PREV_DELAY_0, AluInp.PREV_DELAY_1)
    d[0].pass_through_delay(3, 4)
    d[1].enable_alu(UAluOp.MULTIPLY, AluInp.PREV_DELAY_3, AluInp.PREV_DELAY_4)
    d[1].enable_delay_from_src(DelayInp.PREV_ALU_OUT, 0)          # a
    d[2].enable_alu(UAluOp.ADD, AluInp.PREV_ALU_OUT, AluInp.PREV_DELAY_0)  # s=b+a
    d[2].enable_delay_from_src(DelayInp.PREV_ALU_OUT, 1)          # b
    d[3].enable_alu(UAluOp.ADD, AluInp.CURR_ALU_OUT, AluInp.PREV_ALU_OUT)  # p+=s
    d[3].pass_through_delay(1)
    d[4].enable_alu(UAluOp.SUBTRACT, AluInp.PREV_ALU_OUT, AluInp.PREV_DELAY_1)  # lo=p-b
    d[4].enable_delay_from_src(DelayInp.PREV_ALU_OUT, 2)          # p
    for k in range(5, 8):
        d[k].pass_through_alu()
        d[k].pass_through_delay(2)
    st.require_inp0 = 1
    st.require_inp1 = 1
    st.enable_output(OutSel.ALU_OUT, OutPath.WR0_LO)
    st.enable_output(OutSel.DELAY_2, OutPath.WR0_HI)
    st.trigger = (Trigger.SRC_TENSOR_DONE, Trigger.NONE, Trigger.NONE)
    st.next_uop = (0, 0, 0)
    return [seed, st]


def _register_prefix_mac():
    name = "PREFIX_MAC_GNN"
    if name in dve_ops._SUB_OPCODE_FOR_NAME:
        return next(op for op in dve_ops.OPS if op.name == name)
    spec = Spec(
        body=scan(AluOp.ADD, Src0 * Src1),
        reference=lambda in0, in1, s0, s1, imm2: np.cumsum(
            (in0.astype(np.float32) * in1).reshape(in0.shape[0], -1), axis=-1
        ),
    )
    shas = {}
    row = dve_ops._CUSTOM_DVE_ROW_BASE + len(dve_ops.OPS)
    uops_2x = _prefix_mac_2x_uops()
    specs = {}
    for ver in ("v3", "v4"):
        full = DveOpSpec(name=name, opcode=row, uops=lower(spec, ver=ver),
                         rd1_en=_has_src1(spec), uops_2x=uops_2x, perf_max=1)
        specs[ver] = full
        shas[ver] = full.sha(ver)
    op = dve_ops.DveOp(name, spec, subdim=False, uops_sha=shas)
    # pre-seed the compile cache so table-gen picks up the 2x program
    # (DveOp.compile would rebuild without it)
    for ver, full in specs.items():
        dve_ops._COMPILE_CACHE[(name, ver)] = full
    dve_ops.OPS.append(op)
    dve_ops._SUB_OPCODE_FOR_NAME[name] = row
    dve_ops.CUSTOM_DVE_SPECS[name] = spec
    return op


def _balance_nodes(dst):
    """LPT-pack nodes into (core, local-window) bins to equalize per-bin
    in-degree sums; returns (old->new, new->old) node id permutations."""
    deg = np.bincount(dst, minlength=N).astype(np.int64)
    caps = np.full(N_CORES * N_WIN, WIN, dtype=np.int64)
    caps[N_WIN - 1::N_WIN] = NPC - (N_WIN - 1) * WIN
    order = np.argsort(-deg, kind="stable")
    heap = [(0, b) for b in range(N_CORES * N_WIN)]
    heapq.heapify(heap)
    fill = np.zeros(N_CORES * N_WIN, dtype=np.int64)
    sums = np.zeros(N_CORES * N_WIN, dtype=np.int64)
    members = [[] for _ in range(N_CORES * N_WIN)]
    for v in order:
        while True:
            s, b = heapq.heappop(heap)
            if fill[b] < caps[b]:
                break
        members[b].append(v)
        fill[b] += 1
        sums[b] += deg[v]
        if fill[b] < caps[b]:
            heapq.heappush(heap, (sums[b], b))
    new2old = np.empty(N, dtype=np.int64)
    pos = 0
    for b in range(N_CORES * N_WIN):
        m = members[b]
        new2old[pos:pos + len(m)] = m
        pos += len(m)
    old2new = np.empty(N, dtype=np.int64)
    old2new[new2old] = np.arange(N)
    return old2new, new2old


def _host_prep(n_feat, e_feat, src, dst):
    """Sort edges by dst, shard by dst block, pad each (core, window) edge run
    onto a shared slot grid so the tile->window map is identical on all cores."""
    order = np.argsort(dst, kind="stable")
    src_s, dst_s, ef_s = src[order], dst[order], e_feat[order]

    # per (core, window) counts
    core_e = dst_s // NPC
    loc = dst_s - core_e * NPC
    win_e = loc // WIN
    cnt = np.zeros((N_CORES, N_WIN), dtype=np.int64)
    np.add.at(cnt, (core_e, win_e), 1)

    slot_cnt = cnt.max(axis=0)                       # shared grid
    G = np.concatenate([[0], np.cumsum(slot_cnt)])   # window slot boundaries
    total = int(G[-1])
    T = (total + 127) // 128                         # edge tiles per core
    E_PAD = T * 128

    # per-core padded edge arrays
    src_pad = np.zeros((N_CORES, E_PAD), dtype=np.int32)
    dstrel_pad = np.full((N_CORES, E_PAD), -1.0, dtype=np.float32)
    ef_pad = np.zeros((N_CORES, E_PAD, E_IN), dtype=np.float32)

    # tile -> window band
    w0 = np.zeros(T, dtype=np.int64)       # first window overlapping tile t
    bw = np.zeros(T, dtype=np.int64)       # how many windows overlap tile t
    for t in range(T):
        lo, hi = t * 128, min((t + 1) * 128, total)
        wlo = int(np.searchsorted(G, lo, side="right") - 1)
        whi = int(np.searchsorted(G, max(hi - 1, lo), side="right") - 1)
        wlo, whi = min(wlo, N_WIN - 1), min(whi, N_WIN - 1)
        w0[t] = wlo
        bw[t] = whi - wlo + 1
    B_W = int(bw.max())

    # fill padded arrays: window w of core k occupies slots [G[w], G[w]+cnt[k,w])
    core_starts = np.searchsorted(core_e, np.arange(N_CORES))
    for k in range(N_CORES):
        base = core_starts[k]
        cw = np.concatenate([[0], np.cumsum(cnt[k])])
        for w in range(N_WIN):
            s0, s1 = int(base + cw[w]), int(base + cw[w + 1])
            g0 = int(G[w])
            n_e = s1 - s0
            src_pad[k, g0:g0 + n_e] = src_s[s0:s1]
            ef_pad[k, g0:g0 + n_e] = ef_s[s0:s1]
            # dst_rel relative to the band anchor of the edge's tile
            slots = np.arange(g0, g0 + n_e)
            dstrel_pad[k, g0:g0 + n_e] = (
                loc[s0:s1] - w0[slots // 128] * WIN).astype(np.float32)

    # scatter pair list (t, w) from actual overlap, and per-window tile ranges
    pairs = []
    for t in range(T):
        for j in range(int(bw[t])):
            w = int(w0[t]) + j
            if w < N_WIN:
                pairs.append((t, w))
    win_tiles = {w: [t for (t, ww) in pairs if ww == w] for w in range(N_WIN)}

    # offset of each tile's one-hot block inside the resident oh buffer
    oh_off = np.zeros(T + 1, dtype=np.int64)
    for t in range(T):
        oh_off[t + 1] = oh_off[t] + int(bw[t]) * WIN

    grid = dict(T=T, E_PAD=E_PAD, B_W=B_W, w0=w0, bw=bw, win_tiles=win_tiles,
                oh_off=oh_off)

    import ml_dtypes
    per_core = []
    for k in range(N_CORES):
        per_core.append(dict(
            e_featT=np.ascontiguousarray(ef_pad[k].T).astype(ml_dtypes.bfloat16),  # [10, E_PAD]
            n_featT=np.ascontiguousarray(n_feat[k * NPC:(k + 1) * NPC].T),  # [42, NPC]
            src_idx=np.ascontiguousarray(src_pad[k].reshape(T, 128).T).astype(np.int32),  # [128, T]
            dst_rel=np.ascontiguousarray(dstrel_pad[k].reshape(T, 128).T),  # [128, T]
        ))
    return grid, per_core


def _build_program(grid):
    T, B_W = grid["T"], grid["B_W"]
    w0, bw, win_tiles = grid["w0"], grid["bw"], grid["win_tiles"]
    oh_off = grid["oh_off"]
    OH_TOT = int(oh_off[T])
    PREFIX_MAC = _register_prefix_mac()

    nc = bacc.Bacc("TRN2", target_bir_lowering=False, debug=False,
                   num_devices=N_CORES)

    # ---- kernel I/O ----
    e_featT = nc.dram_tensor("e_featT", [E_IN, grid["E_PAD"]], BF16, kind="ExternalInput")
    n_featT = nc.dram_tensor("n_featT", [D_IN, NPC], F32, kind="ExternalInput")
    src_idx = nc.dram_tensor("src_idx", [128, T], I32, kind="ExternalInput")
    dst_rel = nc.dram_tensor("dst_rel", [128, T], F32, kind="ExternalInput")
    iota = nc.dram_tensor("iota", [128, B_W * WIN], F32, kind="ExternalInput")
    en1_w = nc.dram_tensor("en1_w", [E_IN, E_H], BF16, kind="ExternalInput")
    en1_b = nc.dram_tensor("en1_b", [1, E_H], BF16, kind="ExternalInput")
    en2_wp = nc.dram_tensor("en2_wp", [E_H, NW], BF16, kind="ExternalInput")
    b_r = nc.dram_tensor("b_r", [D_H, D_H], BF16, kind="ExternalInput")
    lin0_wt = nc.dram_tensor("lin0_wt", [D_IN, D_H], F32, kind="ExternalInput")
    lin0_br = nc.dram_tensor("lin0_br", [1, D_H], F32, kind="ExternalInput")
    msgw_top = nc.dram_tensor("msgw_top", [D_H, D_H], F32, kind="ExternalInput")
    msgw_bot = nc.dram_tensor("msgw_bot", [D_H, D_H], F32, kind="ExternalInput")
    msgb_r = nc.dram_tensor("msgb_r", [1, D_H], F32, kind="ExternalInput")
    convb_r = nc.dram_tensor("convb_r", [1, D_H], F32, kind="ExternalInput")
    ident = nc.dram_tensor("ident", [D_H, D_H], F32, kind="ExternalInput")
    ones_r = nc.dram_tensor("ones_r", [1, 128], F32, kind="ExternalInput")
    ones_b = nc.dram_tensor("ones_b", [1, 128], BF16, kind="ExternalInput")
    y = nc.dram_tensor("y", [NPC, D_H], F32, kind="ExternalOutput")

    with tile.TileContext(nc) as tc:
        with (
            tc.tile_pool(name="const", bufs=1) as cpool,
            tc.tile_pool(name="dram", bufs=1, space="DRAM") as dram,
        ):
            # ---- persistent SBUF residents ----
            nfT_sb = cpool.tile([D_IN, NPC], F32)
            srci_sb = cpool.tile([128, T], I32)
            dstr_sb = cpool.tile([128, T], F32)
            iota_sb = cpool.tile([128, B_W * WIN], F32)
            en1w_sb = cpool.tile([E_IN, E_H], BF16)
            en1b_sb = cpool.tile([1, E_H], BF16)
            en2wp_sb = cpool.tile([E_H, NW], BF16)
            br_sb = cpool.tile([D_H, D_H], BF16)
            lin0w_sb = cpool.tile([D_IN, D_H], F32)
            lin0b_sb = cpool.tile([1, D_H], F32)
            mwt_sb = cpool.tile([D_H, D_H], F32)
            mwb_sb = cpool.tile([D_H, D_H], F32)
            mb_sb = cpool.tile([1, D_H], F32)
            cvb_sb = cpool.tile([1, D_H], F32)
            id_sb = cpool.tile([D_H, D_H], F32)
            ones_sb = cpool.tile([1, 128], F32)
            onesb_sb = cpool.tile([1, 128], BF16)
            outT_a = cpool.tile([D_H, NPC], F32)
            outT_b = cpool.tile([D_H, NPC], F32)
            h_all = cpool.tile([128, T * 128], BF16)       # resident h^T tiles
            oh_all = cpool.tile([128, OH_TOT], BF16)       # resident one-hots
            # two prefix-scan halves, each: zero cell + 882 sums + pad
            pfx = cpool.tile([128, 2, HNW + 2], F32)

            for sb, dr in [(nfT_sb, n_featT), (srci_sb, src_idx),
                           (dstr_sb, dst_rel), (iota_sb, iota), (en1w_sb, en1_w),
                           (en1b_sb, en1_b), (en2wp_sb, en2_wp), (br_sb, b_r),
                           (lin0w_sb, lin0_wt), (lin0b_sb, lin0_br), (mwt_sb, msgw_top),
                           (mwb_sb, msgw_bot), (mb_sb, msgb_r), (cvb_sb, convb_r),
                           (id_sb, ident), (ones_sb, ones_r), (onesb_sb, ones_b)]:
                nc.sync.dma_start(sb[:], dr[:])
            nc.gpsimd.memset(pfx[:, :, 0:1], 0.0)

            # ---- DRAM scratch ----
            w_dram = dram.tile([T * 128, HNW], BF16)       # streamed W half
            cc_in = [dram.tile([NPC, D_H], BF16, name=f"cc_in{i}") for i in range(2)]
            cc_out = [dram.tile([N, D_H], BF16, name=f"cc_out{i}", addr_space="Shared")
                      for i in range(STEPS)]

            # ==== setup: h tiles, streamed W half (o-blocks 21..41), one-hots
            ECH = 16  # e_feat tiles per SBUF chunk
            with (
                tc.tile_pool(name="su_e", bufs=2) as su_e,
                tc.tile_pool(name="su_w", bufs=3) as su_w,
                tc.tile_pool(name="su_ph", bufs=2, space="PSUM") as su_ph,
                tc.tile_pool(name="su_pw", bufs=3, space="PSUM") as su_pw,
            ):
                e_ch = None
                for t in range(T):
                    if t % ECH == 0:
                        c0 = t * 128
                        c1 = min((t + ECH) * 128, grid["E_PAD"])
                        e_ch = su_e.tile([E_IN, ECH * 128], BF16, name="e_ch")
                        nc.sync.dma_start(e_ch[:, :c1 - c0], e_featT[:, c0:c1])
                    ph = su_ph.tile([128, 128], F32, name="ph")
                    o = (t % ECH) * 128
                    nc.tensor.matmul(ph[:], lhsT=en1w_sb[:], rhs=e_ch[:, o:o + 128],
                                     start=True, stop=False)
                    nc.tensor.matmul(ph[:], lhsT=en1b_sb[:], rhs=onesb_sb[:1, :],
                                     start=False, stop=True)
                    h_t = h_all[:, t * 128:(t + 1) * 128]
                    nc.scalar.activation(h_t, ph[:],
                                         mybir.ActivationFunctionType.Relu)
                    pw = su_pw.tile([128, 2, 512], F32, name="pw")
                    w_sb = su_w.tile([128, HNW], BF16, name="w_sb")
                    for c in range(2):
                        c0 = (2 + c) * CHUNK
                        nc.tensor.matmul(pw[:, c, :CHUNK], lhsT=h_t,
                                         rhs=en2wp_sb[:, c0:c0 + CHUNK],
                                         start=True, stop=True)
                    # cast fp32 psum -> bf16 sbuf (alternate scalar/vector)
                    dst3 = w_sb[:].rearrange("p (c k) -> p c k", c=2)
                    if t % 2 == 0:
                        nc.scalar.copy(dst3, pw[:, :, :CHUNK])
                    else:
                        nc.vector.tensor_copy(dst3, pw[:, :, :CHUNK])
                    nc.sync.dma_start(w_dram[t * 128:(t + 1) * 128, :], w_sb[:])
                    # one-hot scatter block for this tile (static across steps)
                    bwt = int(bw[t])
                    o0 = int(oh_off[t])
                    nc.gpsimd.tensor_scalar(
                        out=oh_all[:, o0:o0 + bwt * WIN],
                        in0=iota_sb[:, :bwt * WIN],
                        scalar1=dstr_sb[:, t:t + 1],
                        scalar2=None, op0=mybir.AluOpType.is_equal)

            # =========== step pools ===========
            # PSUM (8 banks): rebuilt-W pipeline 2x2; per-window packed pair
            # 2x(1+1): "agt" = msg-aggregate + transpose scratch, "ast" =
            # source-feature sum S + update output.
            with (
                tc.tile_pool(name="st_w", bufs=8) as p_w,
                tc.tile_pool(name="st_m", bufs=8) as p_m,
                tc.tile_pool(name="st_sm", bufs=5) as p_sm,
                tc.tile_pool(name="ps_w", bufs=2, space="PSUM") as ps_w,
                tc.tile_pool(name="ps_win", bufs=2, space="PSUM") as ps_win,
            ):
                def window_cols(w):
                    n0 = w * WIN
                    m = min(WIN, NPC - n0)
                    return n0, m

                def new_window_tiles():
                    agt = ps_win.tile([128, 512], F32, name="agt")
                    ast = ps_win.tile([D_H, 512], F32, name="ast")
                    return agt, ast

                def update_window(w, outT_cur, outT_new, agt, ast, step):
                    """Window epilogue: S@B bias, residual, relu, update matmul,
                    transpose, DMA rows out."""
                    n0, m = window_cols(w)
                    last = step == STEPS
                    aggr = agt[0:D_H, 0:WIN]
                    s_ps = ast[:, 0:WIN]
                    up = ast[:, WIN:2 * WIN]
                    tr = agt[:, WIN:WIN + D_H]
                    # en2 bias via aggregated source features: aggr += (S @ B)^T
                    s_sb = p_sm.tile([D_H, WIN], BF16, name="s_sb")
                    nc.scalar.copy(s_sb[:, :m], s_ps[:, :m])
                    nc.tensor.matmul(aggr[:, :m], lhsT=br_sb[:],
                                     rhs=s_sb[:, :m], start=False, stop=False)
                    # + out (identity residual into conv) and conv bias
                    nc.tensor.matmul(aggr[:, :m], lhsT=id_sb[:],
                                     rhs=outT_cur[:, n0:n0 + m], start=False, stop=False)
                    nc.tensor.matmul(aggr[:, :m], lhsT=cvb_sb[:],
                                     rhs=ones_sb[:1, :m], start=False, stop=True)
                    mT_sb = p_sm.tile([D_H, WIN], F32, name="mT_sb")
                    nc.scalar.activation(mT_sb[:, :m], aggr[:, :m],
                                         mybir.ActivationFunctionType.Relu)
                    nc.tensor.matmul(up[:, :m], lhsT=mwt_sb[:], rhs=mT_sb[:, :m],
                                     start=True, stop=False)
                    nc.tensor.matmul(up[:, :m], lhsT=mwb_sb[:], rhs=outT_cur[:, n0:n0 + m],
                                     start=False, stop=False)
                    nc.tensor.matmul(up[:, :m], lhsT=mb_sb[:], rhs=ones_sb[:1, :m],
                                     start=False, stop=not last)
                    if last:
                        nc.tensor.matmul(up[:, :m], lhsT=id_sb[:], rhs=nfT_sb[:, n0:n0 + m],
                                         start=False, stop=True)
                    nc.scalar.copy(outT_new[:, n0:n0 + m], up[:, :m])
                    nc.tensor.transpose(tr[:m, :], outT_new[:, n0:n0 + m], id_sb[:])
                    if last:
                        rows = p_sm.tile([128, D_H], F32, name="rows_f")
                        nc.scalar.copy(rows[:m, :], tr[:m, :])
                        nc.sync.dma_start(y[n0:n0 + m, :], rows[:m, :])
                    else:
                        rows = p_sm.tile([128, D_H], BF16, name="rows")
                        nc.scalar.copy(rows[:m, :], tr[:m, :])
                        nc.sync.dma_start(cc_in[step % 2][n0:n0 + m, :], rows[:m, :])

                def all_gather(step):
                    nc.gpsimd.collective_compute(
                        "AllGather", mybir.AluOpType.bypass,
                        replica_groups=[list(range(N_CORES))],
                        ins=[cc_in[step % 2].opt()], outs=[cc_out[step].opt()])

                # =========== lin0: out0 = relu(n_feat @ lin0_w + b) ===========
                for w in range(N_WIN):
                    n0, m = window_cols(w)
                    agt, ast = new_window_tiles()
                    up = ast[:, WIN:2 * WIN]
                    tr = agt[:, WIN:WIN + D_H]
                    nc.tensor.matmul(up[:, :m], lhsT=lin0w_sb[:], rhs=nfT_sb[:, n0:n0 + m],
                                     start=True, stop=False)
                    nc.tensor.matmul(up[:, :m], lhsT=lin0b_sb[:], rhs=ones_sb[:1, :m],
                                     start=False, stop=True)
                    nc.scalar.activation(outT_a[:, n0:n0 + m], up[:, :m],
                                         mybir.ActivationFunctionType.Relu)
                    nc.tensor.transpose(tr[:m, :], outT_a[:, n0:n0 + m], id_sb[:])
                    rows = p_sm.tile([128, D_H], BF16, name="rows")
                    nc.scalar.copy(rows[:m, :], tr[:m, :])
                    nc.sync.dma_start(cc_in[0][n0:n0 + m, :], rows[:m, :])
                all_gather(0)

                # =========== message passing steps ===========
                for step in range(1, STEPS + 1):
                    outT_cur = outT_a if step % 2 == 1 else outT_b
                    outT_new = outT_b if step % 2 == 1 else outT_a
                    src_buf = cc_out[step - 1]
                    aggr_of = {}
                    for t in range(T):
                        h_t = h_all[:, t * 128:(t + 1) * 128]
                        # gathered src feats live in cols [42:84) of the
                        # scatter stationary [msg | x]
                        mx = p_m.tile([128, 2 * D_H], BF16, name="mx")
                        nc.gpsimd.indirect_dma_start(
                            out=mx[:, D_H:2 * D_H], out_offset=None, in_=src_buf[:],
                            in_offset=bass.IndirectOffsetOnAxis(
                                ap=srci_sb[:, t:t + 1], axis=0))
                        x_bc = mx[:, D_H:2 * D_H][:, None, :].to_broadcast(
                            [128, O_HALF, D_H])
                        # half a (o-blocks 0..20): rebuild on PE into PSUM
                        wp = ps_w.tile([128, 2, 512], F32, name="wp")
                        for c in range(2):
                            nc.tensor.matmul(wp[:, c, :CHUNK], lhsT=h_t,
                                             rhs=en2wp_sb[:, c * CHUNK:(c + 1) * CHUNK],
                                             start=True, stop=True)
                        nc.vector._custom_dve(
                            PREFIX_MAC, out=pfx[:, 0, 1:HNW + 1],
                            in0=wp[:, :, :CHUNK], in1=x_bc)
                        # half b (o-blocks 21..41): streamed from HBM;
                        # SBUF bf16 source qualifies for the 2x perf mode
                        w_t = p_w.tile([128, HNW], BF16, name="w_t")
                        nc.sync.dma_start(w_t[:], w_dram[t * 128:(t + 1) * 128, :])
                        sc = nc.vector._custom_dve(
                            PREFIX_MAC, out=pfx[:, 1, 1:HNW + 1],
                            in0=w_t[:], in1=x_bc)
                        sc.perf_max = 1
                        nc.vector.tensor_tensor(
                            out=mx[:, 0:D_H].rearrange("p (h o) -> p h o", h=2),
                            in0=pfx[:, :, D_H:HNW + 1:D_H],
                            in1=pfx[:, :, 0:HNW - D_H + 1:D_H],
                            op=mybir.AluOpType.subtract)
                        # scatter matmuls: message aggregate + source-feat sum S
                        bwt = int(bw[t])
                        o0 = int(oh_off[t])
                        for j in range(bwt):
                            w = int(w0[t]) + j
                            if w >= N_WIN:
                                continue
                            tiles_w = win_tiles[w]
                            if w not in aggr_of:
                                aggr_of[w] = new_window_tiles()
                            first = t == tiles_w[0]
                            last_t = t == tiles_w[-1]
                            oh_j = oh_all[:, o0 + j * WIN:o0 + (j + 1) * WIN]
                            agt, ast = aggr_of[w]
                            nc.tensor.matmul(agt[0:D_H, 0:WIN], lhsT=mx[:, 0:D_H],
                                             rhs=oh_j, start=first, stop=False)
                            nc.tensor.matmul(ast[:, 0:WIN], lhsT=mx[:, D_H:2 * D_H],
                                             rhs=oh_j, start=first, stop=last_t)
                            if last_t:
                                update_window(w, outT_cur, outT_new,
                                              agt, ast, step)
                                aggr_of.pop(w)
                    if step < STEPS:
                        all_gather(step)

    nc.compile()
    return nc


_CACHED = {}


def kernel(n_feat, e_feat, src, dst, lin0_w, lin0_b, en1_w, en1_b,
           en2_w, en2_b, conv_bias, msg_w, msg_b):
    import ml_dtypes
    n_feat = np.asarray(n_feat, dtype=np.float32)
    e_feat = np.asarray(e_feat, dtype=np.float32)
    src = np.asarray(src, dtype=np.int32)
    dst = np.asarray(dst, dtype=np.int32)

    # balance (core, window) loads via a global node relabeling
    old2new, new2old = _balance_nodes(dst)
    n_feat_p = np.ascontiguousarray(n_feat[new2old])
    src_p = old2new[src].astype(np.int32)
    dst_p = old2new[dst].astype(np.int32)

    grid, per_core = _host_prep(n_feat_p, e_feat, src_p, dst_p)

    key = (grid["T"], grid["B_W"], tuple(grid["w0"].tolist()))
    if key not in _CACHED:
        _CACHED.clear()
        _CACHED[key] = _build_program(grid)
    nc = _CACHED[key]

    # en2_w reshaped so W' columns are (o, i) o-major, matching the scan's
    # per-o prefix-difference extraction
    en2_wp = np.ascontiguousarray(
        np.asarray(en2_w, np.float32).reshape(E_H, D_H, D_H).transpose(0, 2, 1)
        .reshape(E_H, NW)).astype(ml_dtypes.bfloat16)
    shared = dict(
        iota=np.tile(np.arange(grid["B_W"] * WIN, dtype=np.float32), (128, 1)),
        en1_w=np.asarray(en1_w, np.float32).astype(ml_dtypes.bfloat16),
        en1_b=np.asarray(en1_b, np.float32).reshape(1, E_H).astype(ml_dtypes.bfloat16),
        en2_wp=en2_wp,
        b_r=np.ascontiguousarray(
            np.asarray(en2_b, np.float32).reshape(D_H, D_H)).astype(ml_dtypes.bfloat16),
        lin0_wt=np.asarray(lin0_w, np.float32),
        lin0_br=np.asarray(lin0_b, np.float32).reshape(1, D_H),
        msgw_top=np.ascontiguousarray(np.asarray(msg_w, np.float32)[:D_H, :]),
        msgw_bot=np.ascontiguousarray(np.asarray(msg_w, np.float32)[D_H:, :]),
        msgb_r=np.asarray(msg_b, np.float32).reshape(1, D_H),
        convb_r=np.asarray(conv_bias, np.float32).reshape(1, D_H),
        ident=np.eye(D_H, dtype=np.float32),
        ones_r=np.ones((1, 128), dtype=np.float32),
        ones_b=np.ones((1, 128), dtype=ml_dtypes.bfloat16),
    )
    in_maps = []
    for k in range(N_CORES):
        m = dict(shared)
        m.update(per_core[k])
        in_maps.append(m)

    res = bass_utils.run_bass_kernel_spmd(nc, in_maps, core_ids=list(range(N_CORES)))
    out_p = np.concatenate([res.results[k]["y"] for k in range(N_CORES)], axis=0)
    out = np.empty_like(out_p)
    out[new2old] = out_p                      # undo the node relabeling
    return out.astype(np.float32)
